# revision 19
# baseline (speedup 1.0000x reference)
"""Single-head self-attention (B=8, S=2048, D=1024) on 8 TRN2 NeuronCores.

Data-parallel over batch: core b computes attention for x[b].
All compute in bf16 matmuls with fp32 PSUM accumulation; softmax in fp32.

Prologue avoids the DRAM round-trip cast: x streams fp32 over HWDGE in
row-chunks, DVE casts to bf16, the (otherwise idle) PE transposes each
chunk's eight 128x128 tiles into a single PSUM bank, and the scalar
engine copies them out to the persistent x^T. Weights other than Wq load
via SWDGE cast; Wq (on the critical path) streams fp32 over HWDGE with a
DVE cast.
"""

import sys

sys.path.insert(0, "/opt/trn_rl_repo")

import numpy as np

B, S, D = 8, 2048, 1024
P = 128
SO = S // P  # 16 s-tiles
DO = D // P  # 8 d-tiles
IC = 512  # i-chunk (query chunk) width
NIC = S // IC  # 4
NF = D // 512  # 2 free-dim chunks for D-wide outputs
NCH = S // P  # 16 x row-chunks

_CACHE = {}


def _emit_body(nc, tc, t):
    import concourse.mybir as mybir
    from concourse.masks import make_identity

    F32 = mybir.dt.float32
    BF16 = mybir.dt.bfloat16
    Exp = mybir.ActivationFunctionType.Exp
    Ident = mybir.ActivationFunctionType.Identity

    const = tc.alloc_tile_pool(name="const", bufs=1)
    dram = tc.alloc_tile_pool(name="dram", bufs=1, space="DRAM")

    # ---- weights, all on SWDGE (cast fp32->bf16 in the DMA). Wq first and
    # alone (it gates the first projections); fences keep later weights from
    # stealing SWDGE bandwidth via round-robin.
    wqkv = tc.alloc_tile_pool(name="wqkv", bufs=1)
    W_sb = {}
    for name in ("Wq", "Wk", "Wv"):
        W_sb[name] = wqkv.tile([P, DO, D], BF16, name=f"{name}_sb")

    def load_w(name):
        nc.gpsimd.dma_start(
            W_sb[name][:], t[name].rearrange("(ko ki) e -> ki ko e", ki=P)
        )

    load_w("Wq")

    # identity for PE transposes (gpsimd compute overlaps the Wq DMA);
    # fp32 to match the fp32 x chunks it transposes
    identity = const.tile([P, P], F32, name="ident")
    make_identity(nc, identity[:])

    fence_d = dram.tile([2, 16], BF16, name="fence_d")
    nc.gpsimd.dma_start(fence_d[0:1, :], W_sb["Wq"][0:1, 7, 1008:1024])
    load_w("Wk")
    load_w("Wv")
    nc.gpsimd.dma_start(fence_d[1:2, :], W_sb["Wv"][0:1, 7, 1008:1024])

    ones_row = const.tile([1, P], BF16, name="ones_row")
    nc.vector.memset(ones_row[:], 1.0)
    ones_j = const.tile([P, 1], BF16, name="ones_j")
    nc.vector.memset(ones_j[:], 1.0)

    # per-partition biases for Q^T/K^T (e on partitions); tiny loads on the
    # scalar HWDGE queue ahead of the x chunks.
    bq_sb = const.tile([P, DO], F32, name="bq_sb")
    nc.scalar.dma_start(bq_sb[:], t["bq"].rearrange("(eo ei) -> ei eo", ei=P))
    bk_sb = const.tile([P, DO], F32, name="bk_sb")
    nc.scalar.dma_start(bk_sb[:], t["bk"].rearrange("(eo ei) -> ei eo", ei=P))

    # ---- persistent activations
    xt_pool = tc.alloc_tile_pool(name="xt_pool", bufs=1)
    xT = xt_pool.tile([P, DO, S], BF16, name="xT")  # [d_inner, d_outer, s]
    QT = const.tile([P, DO, S], BF16, name="QT")  # [e_i, e_o, s]
    KT = const.tile([P, DO, S], BF16, name="KT")
    V = const.tile([P, SO, D], BF16, name="V")  # [s_i, s_o, e]
    recip_sb = const.tile([P, SO], F32, name="recip_sb")

    # bv_* only live until the V drains; park them in xt_pool so they free
    # with the projections.
    bv_row = xt_pool.tile([1, D], F32, name="bv_row")
    nc.scalar.dma_start(bv_row[:], t["bv"].rearrange("(a d) -> a d", a=1))
    bv_bcast = xt_pool.tile([P, D], F32, name="bv_bcast")
    bv_row_bf = xt_pool.tile([1, D], BF16, name="bv_row_bf")

    def emit_qk_proj(Wn, b_sb, OUT, sc, ppsum):
        # lhsT = W tile [d, e-tile] (stationary), rhs = xT [d, s-chunk]
        for eo in range(DO):
            pp = ppsum.tile([P, 512], F32, tag="proj", name="pp")
            for k in range(DO):
                nc.tensor.matmul(
                    pp[:],
                    W_sb[Wn][:, k, eo * P : (eo + 1) * P],
                    xT[:, k, sc * 512 : (sc + 1) * 512],
                    start=(k == 0),
                    stop=(k == DO - 1),
                )
            nc.scalar.activation(
                OUT[:, eo, sc * 512 : (sc + 1) * 512],
                pp[:],
                Ident,
                bias=b_sb[:, eo : eo + 1],
            )

    # ---- prologue: stream x fp32 over both HWDGE queues, DVE-cast to bf16,
    # PE-transpose each chunk's 8 k-tiles into one PSUM bank. Q-proj s-chunks
    # are interleaved so the PE starts projecting as soon as Wq + the first
    # 4 chunks are in.
    with tc.tile_pool(name="stage", bufs=5) as stage, \
         tc.tile_pool(name="tpsum", bufs=2, space="PSUM") as tpsum, \
         tc.tile_pool(name="ppsum", bufs=4, space="PSUM") as ppsum:
        # all x chunks on the (otherwise idle) sync queue; the in-order queue
        # plus stage-pool WAR deps pipeline the stream 5 chunks deep. The PE
        # transposes fp32 directly (2 cy/row) and the scalar copy casts
        # PSUM fp32 -> SBUF bf16, so no DVE work gates the stream.
        for c in range(NCH):
            st = stage.tile([P, D], F32, tag="xs", name="xs")
            nc.sync.dma_start(st[:], t["x"][c * P : (c + 1) * P, :])
            # 8 transposes into two PSUM banks as a single accumulation
            # group (disjoint byte ranges; one zero-region mark each).
            ps = tpsum.tile([P, DO, P], F32, tag="tp", name="tp")
            for j in range(DO):
                nc.tensor.matmul(
                    ps[:, j, :],
                    st[:, j * P : (j + 1) * P],
                    identity[:],
                    start=(j == 0 or j == 4),
                    stop=(j == 3 or j == DO - 1),
                    is_transpose=True,
                )
            nc.scalar.copy(xT[:, :, c * P : (c + 1) * P], ps[:])
            if c % 4 == 3:
                emit_qk_proj("Wq", bq_sb, QT, c // 4, ppsum)

        for sc in range(4):
            emit_qk_proj("Wk", bk_sb, KT, sc, ppsum)

    with tc.tile_pool(name="vpsum", bufs=4, space="PSUM") as vpsum:
        # bv broadcast (bf16 rhs so the tiny matmuls run at full rate);
        # emitted after Q/K so its late DVE cast can't stall the PE stream.
        nc.vector.tensor_copy(bv_row_bf[:], bv_row[:])
        for fc in range(NF):
            psb = vpsum.tile([P, 512], F32, tag="proj", name="bps")
            nc.tensor.matmul(
                psb[:], ones_row[:], bv_row_bf[:, fc * 512 : (fc + 1) * 512],
                start=True, stop=True,
            )
            nc.vector.tensor_copy(bv_bcast[:, fc * 512 : (fc + 1) * 512], psb[:])

        # V: lhsT = xT tile [d, s-tile], rhs = Wv [d, e-chunk]
        for so in range(SO):
            pss = [
                vpsum.tile([P, 512], F32, tag="proj", name=f"pv{fc}")
                for fc in range(NF)
            ]
            for k in range(DO):
                for fc in range(NF):
                    nc.tensor.matmul(
                        pss[fc][:],
                        xT[:, k, so * P : (so + 1) * P],
                        W_sb["Wv"][:, k, fc * 512 : (fc + 1) * 512],
                        start=(k == 0),
                        stop=(k == DO - 1),
                    )
            for fc in range(NF):
                nc.vector.tensor_add(
                    V[:, so, fc * 512 : (fc + 1) * 512],
                    pss[fc][:],
                    bv_bcast[:, fc * 512 : (fc + 1) * 512],
                )

    # projections done: reclaim Wq/Wk/Wv and xT space for YT, Wo
    xt_pool.release()
    wqkv.release()

    late = tc.alloc_tile_pool(name="late", bufs=1)
    YT = late.tile([P, DO, S], BF16, name="YT")  # [e_i, e_o, i]
    Wo_sb = late.tile([P, DO, D], BF16, name="Wo_sb")
    nc.gpsimd.dma_start(
        Wo_sb[:], t["Wo"].rearrange("(ko ki) e -> ki ko e", ki=P)
    )
    bo_bcast = late.tile([P, D], F32, name="bo_bcast")
    bo_row = late.tile([1, D], F32, name="bo_row")
    nc.scalar.dma_start(bo_row[:], t["bo"].rearrange("(a d) -> a d", a=1))
    bo_row_bf = late.tile([1, D], BF16, name="bo_row_bf")
    nc.vector.tensor_copy(bo_row_bf[:], bo_row[:])
    with tc.tile_pool(name="bopsum", bufs=2, space="PSUM") as bopsum:
        for fc in range(NF):
            psb = bopsum.tile([P, 512], F32, tag="bo", name="bops")
            nc.tensor.matmul(
                psb[:], ones_row[:], bo_row_bf[:, fc * 512 : (fc + 1) * 512],
                start=True, stop=True,
            )
            nc.vector.tensor_copy(bo_bcast[:, fc * 512 : (fc + 1) * 512], psb[:])

    cs_dram = dram.tile([S], F32)
    cs_dram_2d = cs_dram.rearrange("(a s) -> a s", a=1)

    # ---- attention. PE stream interleave: S(0) S(1) cs(0) Y(0) S(2)
    # cs(1) Y(1) S(3) cs(2) Y(2) cs(3) Y(3) — Y(ic)'s dependency on all of
    # E(ic) is hidden behind S(ic+1), so the PE never waits on exp.
    inv_sqrt_d = float(1.0 / np.sqrt(D))
    with tc.tile_pool(name="epool", bufs=2) as epool, \
         tc.tile_pool(name="red_pool", bufs=1) as red_pool, \
         tc.tile_pool(name="csb_pool", bufs=2) as csb_pool, \
         tc.tile_pool(name="spsum", bufs=3, space="PSUM") as spsum, \
         tc.tile_pool(name="cpsum", bufs=2, space="PSUM") as cpsum, \
         tc.tile_pool(name="ypsum", bufs=3, space="PSUM") as ypsum:
        E = {}

        def emit_scores(ic):
            isl = slice(ic * IC, (ic + 1) * IC)
            E[ic] = epool.tile([P, SO, IC], BF16, tag="E", name="E")
            for jt in range(SO):
                ps = spsum.tile([P, IC], F32, tag="S", name="sps")
                for k in range(DO):
                    nc.tensor.matmul(
                        ps[:],
                        KT[:, k, jt * P : (jt + 1) * P],
                        QT[:, k, isl],
                        start=(k == 0),
                        stop=(k == DO - 1),
                    )
                nc.scalar.activation(
                    E[ic][:, jt, :], ps[:], Exp, scale=inv_sqrt_d
                )

        def emit_cs(ic):
            # softmax denominators: DVE folds 16 E planes down to 4, then
            # 4 ones-matmuls finish the partition reduction.
            isl = slice(ic * IC, (ic + 1) * IC)
            red = red_pool.tile([P, 12, IC], BF16, tag="red", name="red")
            nc.vector.tensor_add(
                red[:, 0:8, :], E[ic][:, 0:8, :], E[ic][:, 8:16, :]
            )
            nc.vector.tensor_add(
                red[:, 8:12, :], red[:, 0:4, :], red[:, 4:8, :]
            )
            cs = cpsum.tile([1, IC], F32, tag="cs", name="cs")
            for tt in range(4):
                nc.tensor.matmul(
                    cs[:], ones_j[:], red[:, 8 + tt, :],
                    start=(tt == 0), stop=(tt == 3),
                )
            csb = csb_pool.tile([1, IC], F32, tag="csb", name="csb")
            nc.vector.tensor_copy(csb[:], cs[:])
            nc.sync.dma_start(cs_dram_2d[:, isl], csb[:])

        def emit_Y(ic):
            isl = slice(ic * IC, (ic + 1) * IC)
            for eo in range(DO):
                py = ypsum.tile([P, IC], F32, tag="Y", name="yps")
                for jt in range(SO):
                    nc.tensor.matmul(
                        py[:],
                        V[:, jt, eo * P : (eo + 1) * P],
                        E[ic][:, jt, :],
                        start=(jt == 0),
                        stop=(jt == SO - 1),
                    )
                nc.vector.tensor_copy(YT[:, eo, isl], py[:])

        emit_scores(0)
        emit_scores(1)
        emit_cs(0)
        emit_Y(0)
        emit_scores(2)
        emit_cs(1)
        emit_Y(1)
        emit_scores(3)
        emit_cs(2)
        emit_Y(2)
        emit_cs(3)
        emit_Y(3)

    # reshape colsum [S] in DRAM -> [128, SO] (per-partition for output stage)
    nc.sync.dma_start(recip_sb[:], cs_dram.rearrange("(io ii) -> ii io", ii=P))
    nc.vector.reciprocal(recip_sb[:], recip_sb[:])

    # ---- output projection: out = (Y^T.T @ Wo) * recip + bo
    out_r = t["out"].rearrange("(so si) f -> si so f", si=P)
    with tc.tile_pool(name="opool", bufs=3) as opool, \
         tc.tile_pool(name="opsum", bufs=4, space="PSUM") as opsum:
        for it in range(SO):
            pss = [
                opsum.tile([P, 512], F32, tag="O", name=f"po{fc}")
                for fc in range(NF)
            ]
            for k in range(DO):
                for fc in range(NF):
                    nc.tensor.matmul(
                        pss[fc][:],
                        YT[:, k, it * P : (it + 1) * P],
                        Wo_sb[:, k, fc * 512 : (fc + 1) * 512],
                        start=(k == 0),
                        stop=(k == DO - 1),
                    )
            o_sb = opool.tile([P, D], F32, tag="osb", name="o_sb")
            for fc in range(NF):
                fsl = slice(fc * 512, (fc + 1) * 512)
                # scale on the scalar engine, bias-add on DVE
                nc.scalar.mul(o_sb[:, fsl], pss[fc][:], recip_sb[:, it : it + 1])
                nc.vector.tensor_add(o_sb[:, fsl], o_sb[:, fsl], bo_bcast[:, fsl])
            eng = nc.sync if it % 2 == 0 else nc.scalar
            eng.dma_start(out_r[:, it, :], o_sb[:])

    late.release()
    dram.release()
    const.release()


def _build():
    if "nc" in _CACHE:
        return _CACHE["nc"]
    import concourse.tile as tile
    import concourse.mybir as mybir
    from concourse import bacc

    nc = bacc.Bacc("TRN2", target_bir_lowering=False, debug=False, num_devices=8)
    F32 = mybir.dt.float32
    t = {}
    t["x"] = nc.dram_tensor("x", [S, D], F32, kind="ExternalInput").ap()
    for name in ("Wq", "Wk", "Wv", "Wo"):
        t[name] = nc.dram_tensor(name, [D, D], F32, kind="ExternalInput").ap()
    for name in ("bq", "bk", "bv", "bo"):
        t[name] = nc.dram_tensor(name, [D], F32, kind="ExternalInput").ap()
    t["out"] = nc.dram_tensor("out", [S, D], F32, kind="ExternalOutput").ap()

    with tile.TileContext(nc) as tc:
        _emit_body(nc, tc, t)
    nc.compile()
    _CACHE["nc"] = nc
    return nc


def kernel(x, Wq, bq, Wk, bk, Wv, bv, Wo, bo, _trace=False):
    from concourse.bass_utils import run_bass_kernel_spmd

    nc = _build()
    x = np.ascontiguousarray(np.asarray(x, dtype=np.float32))
    shared = {
        "Wq": np.ascontiguousarray(np.asarray(Wq, dtype=np.float32)),
        "Wk": np.ascontiguousarray(np.asarray(Wk, dtype=np.float32)),
        "Wv": np.ascontiguousarray(np.asarray(Wv, dtype=np.float32)),
        "Wo": np.ascontiguousarray(np.asarray(Wo, dtype=np.float32)),
        "bq": np.ascontiguousarray(np.asarray(bq, dtype=np.float32)),
        "bk": np.ascontiguousarray(np.asarray(bk, dtype=np.float32)),
        "bv": np.ascontiguousarray(np.asarray(bv, dtype=np.float32)),
        "bo": np.ascontiguousarray(np.asarray(bo, dtype=np.float32)),
    }
    in_maps = [{"x": x[b], **shared} for b in range(B)]
    res = run_bass_kernel_spmd(
        nc, in_maps, core_ids=list(range(B)), trace=_trace
    )
    out = np.stack([r["out"] for r in res.results], axis=0)
    if _trace:
        return out, res
    return out


# revision 26
# speedup vs baseline: 1.1997x; 1.1997x over previous
"""Single-head self-attention (B=8, S=2048, D=1024) on 8 TRN2 NeuronCores.

Data-parallel over batch: core b computes attention for x[b].
All compute in bf16 matmuls with fp32 PSUM accumulation; softmax in fp32.

Prologue avoids the DRAM round-trip cast: x streams fp32 over HWDGE in
row-chunks, DVE casts to bf16, the (otherwise idle) PE transposes each
chunk's eight 128x128 tiles into a single PSUM bank, and the scalar
engine copies them out to the persistent x^T. Weights other than Wq load
via SWDGE cast; Wq (on the critical path) streams fp32 over HWDGE with a
DVE cast.
"""

import sys

sys.path.insert(0, "/opt/trn_rl_repo")

import numpy as np

B, S, D = 8, 2048, 1024
P = 128
SO = S // P  # 16 s-tiles
DO = D // P  # 8 d-tiles
IC = 512  # i-chunk (query chunk) width
NIC = S // IC  # 4
NF = D // 512  # 2 free-dim chunks for D-wide outputs
NCH = S // P  # 16 x row-chunks

_CACHE = {}


def _emit_body(nc, tc, t):
    import concourse.mybir as mybir
    from concourse.masks import make_identity

    F32 = mybir.dt.float32
    BF16 = mybir.dt.bfloat16
    Exp = mybir.ActivationFunctionType.Exp
    Ident = mybir.ActivationFunctionType.Identity

    const = tc.alloc_tile_pool(name="const", bufs=1)
    dram = tc.alloc_tile_pool(name="dram", bufs=1, space="DRAM")

    # ---- weights, all on SWDGE (cast fp32->bf16 in the DMA). Wq first and
    # alone (it gates the first projections); fences keep later weights from
    # stealing SWDGE bandwidth via round-robin.
    wqkv = tc.alloc_tile_pool(name="wqkv", bufs=1)
    W_sb = {}
    for name in ("Wq", "Wk", "Wv"):
        W_sb[name] = wqkv.tile([P, DO, D], BF16, name=f"{name}_sb")

    def load_w(name):
        nc.gpsimd.dma_start(
            W_sb[name][:], t[name].rearrange("(ko ki) e -> ki ko e", ki=P)
        )

    load_w("Wq")

    # identity for PE transposes (gpsimd compute overlaps the Wq DMA)
    identity = const.tile([P, P], BF16, name="ident")
    make_identity(nc, identity[:])

    # bias rows pre-cast to bf16 by tiny SWDGE cast-DMAs (keeps the DVE
    # stream free of casts whose inputs arrive late)
    bv_row_bf = const.tile([1, D], BF16, name="bv_row_bf")
    nc.gpsimd.dma_start(bv_row_bf[:], t["bv"].rearrange("(a d) -> a d", a=1))

    fence_d = dram.tile([2, 16], BF16, name="fence_d")
    nc.gpsimd.dma_start(fence_d[0:1, :], W_sb["Wq"][0:1, 7, 1008:1024])
    load_w("Wk")
    load_w("Wv")
    nc.gpsimd.dma_start(fence_d[1:2, :], W_sb["Wv"][0:1, 7, 1008:1024])

    ones_row = const.tile([1, P], BF16, name="ones_row")
    nc.vector.memset(ones_row[:], 1.0)
    ones_j = const.tile([P, 1], BF16, name="ones_j")
    nc.vector.memset(ones_j[:], 1.0)

    # per-partition biases for Q^T/K^T (e on partitions); tiny loads on the
    # scalar HWDGE queue ahead of the x chunks.
    bq_sb = const.tile([P, DO], F32, name="bq_sb")
    nc.scalar.dma_start(bq_sb[:], t["bq"].rearrange("(eo ei) -> ei eo", ei=P))
    bk_sb = const.tile([P, DO], F32, name="bk_sb")
    nc.scalar.dma_start(bk_sb[:], t["bk"].rearrange("(eo ei) -> ei eo", ei=P))

    # ---- persistent activations
    xt_pool = tc.alloc_tile_pool(name="xt_pool", bufs=1)
    xT = xt_pool.tile([P, DO, S], BF16, name="xT")  # [d_inner, d_outer, s]
    QT = const.tile([P, DO, S], BF16, name="QT")  # [e_i, e_o, s]
    KT = const.tile([P, DO, S], BF16, name="KT")
    V = const.tile([P, SO, D], BF16, name="V")  # [s_i, s_o, e]
    recip_sb = const.tile([P, SO], F32, name="recip_sb")

    # bv_bcast only lives until the V drains; park it in xt_pool so it
    # frees with the projections.
    bv_bcast = xt_pool.tile([P, D], F32, name="bv_bcast")

    def emit_qk_proj(Wn, b_sb, OUT, sc, ppsum):
        # lhsT = W tile [d, e-tile] (stationary), rhs = xT [d, s-chunk]
        for eo in range(DO):
            pp = ppsum.tile([P, 512], F32, tag="proj", name="pp")
            for k in range(DO):
                nc.tensor.matmul(
                    pp[:],
                    W_sb[Wn][:, k, eo * P : (eo + 1) * P],
                    xT[:, k, sc * 512 : (sc + 1) * 512],
                    start=(k == 0),
                    stop=(k == DO - 1),
                )
            nc.scalar.activation(
                OUT[:, eo, sc * 512 : (sc + 1) * 512],
                pp[:],
                Ident,
                bias=b_sb[:, eo : eo + 1],
            )

    # ---- prologue: stream x fp32 over both HWDGE queues, DVE-cast to bf16,
    # PE-transpose each chunk's 8 k-tiles into one PSUM bank. Q-proj s-chunks
    # are interleaved so the PE starts projecting as soon as Wq + the first
    # 4 chunks are in.
    with tc.tile_pool(name="stage", bufs=3) as stage, \
         tc.tile_pool(name="bstage", bufs=4) as bstage, \
         tc.tile_pool(name="tpsum", bufs=2, space="PSUM") as tpsum, \
         tc.tile_pool(name="ppsum", bufs=4, space="PSUM") as ppsum:
        # all x chunks on the (otherwise idle) sync queue; the in-order queue
        # plus stage-pool WAR deps pipeline the stream 3 chunks deep without
        # ever blocking the scalar engine on a dispatch
        for c in range(NCH):
            st = stage.tile([P, D], F32, tag="xs", name="xs")
            nc.sync.dma_start(st[:], t["x"][c * P : (c + 1) * P, :])
            bt = bstage.tile([P, D], BF16, tag="xb", name="xb")
            nc.vector.tensor_copy(bt[:], st[:])  # fp32 -> bf16
            # 8 transposes into one PSUM bank as a single accumulation
            # group (disjoint byte ranges; one zero-region mark).
            ps = tpsum.tile([P, DO, P], BF16, tag="tp", name="tp")
            for j in range(DO):
                nc.tensor.matmul(
                    ps[:, j, :],
                    bt[:, j * P : (j + 1) * P],
                    identity[:],
                    start=(j == 0),
                    stop=(j == DO - 1),
                    is_transpose=True,
                )
            eng_copy = (
                nc.scalar.copy if c % 2 == 0 else nc.vector.tensor_copy
            )
            eng_copy(xT[:, :, c * P : (c + 1) * P], ps[:])
            if c % 4 == 3:
                emit_qk_proj("Wq", bq_sb, QT, c // 4, ppsum)

        for sc in range(4):
            emit_qk_proj("Wk", bk_sb, KT, sc, ppsum)

    with tc.tile_pool(name="vpsum", bufs=4, space="PSUM") as vpsum:
        # bv broadcast (bf16 rhs so the tiny matmuls run at full rate)
        for fc in range(NF):
            psb = vpsum.tile([P, 512], F32, tag="proj", name="bps")
            nc.tensor.matmul(
                psb[:], ones_row[:], bv_row_bf[:, fc * 512 : (fc + 1) * 512],
                start=True, stop=True,
            )
            nc.vector.tensor_copy(bv_bcast[:, fc * 512 : (fc + 1) * 512], psb[:])

        # V: lhsT = xT tile [d, s-tile], rhs = Wv [d, e-chunk]
        for so in range(SO):
            pss = [
                vpsum.tile([P, 512], F32, tag="proj", name=f"pv{fc}")
                for fc in range(NF)
            ]
            for k in range(DO):
                for fc in range(NF):
                    nc.tensor.matmul(
                        pss[fc][:],
                        xT[:, k, so * P : (so + 1) * P],
                        W_sb["Wv"][:, k, fc * 512 : (fc + 1) * 512],
                        start=(k == 0),
                        stop=(k == DO - 1),
                    )
            for fc in range(NF):
                nc.vector.tensor_add(
                    V[:, so, fc * 512 : (fc + 1) * 512],
                    pss[fc][:],
                    bv_bcast[:, fc * 512 : (fc + 1) * 512],
                )

    # projections done: reclaim Wq/Wk/Wv and xT space for YT, Wo
    xt_pool.release()
    wqkv.release()

    late = tc.alloc_tile_pool(name="late", bufs=1)
    YT = late.tile([P, DO, S], BF16, name="YT")  # [e_i, e_o, i]
    Wo_sb = late.tile([P, DO, D], BF16, name="Wo_sb")
    bo_row_bf = late.tile([1, D], BF16, name="bo_row_bf")
    nc.gpsimd.dma_start(bo_row_bf[:], t["bo"].rearrange("(a d) -> a d", a=1))
    nc.gpsimd.dma_start(
        Wo_sb[:], t["Wo"].rearrange("(ko ki) e -> ki ko e", ki=P)
    )
    bo_bcast = late.tile([P, D], F32, name="bo_bcast")

    cs_dram = dram.tile([S], F32)
    cs_dram_2d = cs_dram.rearrange("(a s) -> a s", a=1)
    cs_dram_cols = cs_dram.rearrange("(io ii) -> ii io", ii=P)

    # ---- attention. PE stream interleave: S(0) S(1) cs(0) Y(0) S(2)
    # cs(1) Y(1) S(3) cs(2) Y(2) cs(3) Y(3) — Y(ic)'s dependency on all of
    # E(ic) is hidden behind S(ic+1), so the PE never waits on exp.
    inv_sqrt_d = float(1.0 / np.sqrt(D))
    with tc.tile_pool(name="epool", bufs=2) as epool, \
         tc.tile_pool(name="red_pool", bufs=1) as red_pool, \
         tc.tile_pool(name="csb_pool", bufs=2) as csb_pool, \
         tc.tile_pool(name="spsum", bufs=3, space="PSUM") as spsum, \
         tc.tile_pool(name="cpsum", bufs=2, space="PSUM") as cpsum, \
         tc.tile_pool(name="ypsum", bufs=3, space="PSUM") as ypsum:
        E = {}

        def emit_scores(ic):
            isl = slice(ic * IC, (ic + 1) * IC)
            E[ic] = epool.tile([P, SO, IC], BF16, tag="E", name="E")
            for jt in range(SO):
                ps = spsum.tile([P, IC], F32, tag="S", name="sps")
                for k in range(DO):
                    nc.tensor.matmul(
                        ps[:],
                        KT[:, k, jt * P : (jt + 1) * P],
                        QT[:, k, isl],
                        start=(k == 0),
                        stop=(k == DO - 1),
                    )
                nc.scalar.activation(
                    E[ic][:, jt, :], ps[:], Exp, scale=inv_sqrt_d
                )

        def emit_cs(ic):
            # softmax denominators: DVE folds 16 E planes down to 4, then
            # 4 ones-matmuls finish the partition reduction.
            isl = slice(ic * IC, (ic + 1) * IC)
            red = red_pool.tile([P, 12, IC], BF16, tag="red", name="red")
            nc.vector.tensor_add(
                red[:, 0:8, :], E[ic][:, 0:8, :], E[ic][:, 8:16, :]
            )
            nc.vector.tensor_add(
                red[:, 8:12, :], red[:, 0:4, :], red[:, 4:8, :]
            )
            cs = cpsum.tile([1, IC], F32, tag="cs", name="cs")
            for tt in range(4):
                nc.tensor.matmul(
                    cs[:], ones_j[:], red[:, 8 + tt, :],
                    start=(tt == 0), stop=(tt == 3),
                )
            csb = csb_pool.tile([1, IC], F32, tag="csb", name="csb")
            nc.vector.tensor_copy(csb[:], cs[:])
            nc.sync.dma_start(cs_dram_2d[:, isl], csb[:])
            # round-trip this chunk's denominators to per-partition layout
            # and invert now, so the output drains never wait on recip
            nc.sync.dma_start(
                recip_sb[:, ic * 4 : (ic + 1) * 4],
                cs_dram_cols[:, ic * 4 : (ic + 1) * 4],
            )
            nc.vector.reciprocal(
                recip_sb[:, ic * 4 : (ic + 1) * 4],
                recip_sb[:, ic * 4 : (ic + 1) * 4],
            )

        def emit_Y(ic):
            isl = slice(ic * IC, (ic + 1) * IC)
            for eo in range(DO):
                py = ypsum.tile([P, IC], F32, tag="Y", name="yps")
                for jt in range(SO):
                    nc.tensor.matmul(
                        py[:],
                        V[:, jt, eo * P : (eo + 1) * P],
                        E[ic][:, jt, :],
                        start=(jt == 0),
                        stop=(jt == SO - 1),
                    )
                nc.vector.tensor_copy(YT[:, eo, isl], py[:])

        emit_scores(0)
        # bo broadcast, tucked behind S(0) so neither the PE nor DVE waits
        # (bo_row_bf lands off the SWDGE queue long before this)
        for fc in range(NF):
            psb = spsum.tile([P, IC], F32, tag="S", name="bops")
            nc.tensor.matmul(
                psb[:], ones_row[:], bo_row_bf[:, fc * 512 : (fc + 1) * 512],
                start=True, stop=True,
            )
            nc.vector.tensor_copy(bo_bcast[:, fc * 512 : (fc + 1) * 512], psb[:])
        emit_scores(1)
        emit_cs(0)
        emit_Y(0)
        emit_scores(2)
        emit_cs(1)
        emit_Y(1)
        emit_scores(3)
        emit_cs(2)
        emit_Y(2)
        emit_cs(3)
        emit_Y(3)

    # ---- output projection: out = (Y^T.T @ Wo) * recip + bo
    out_r = t["out"].rearrange("(so si) f -> si so f", si=P)
    with tc.tile_pool(name="opool", bufs=3) as opool, \
         tc.tile_pool(name="opsum", bufs=4, space="PSUM") as opsum:
        for it in range(SO):
            pss = [
                opsum.tile([P, 512], F32, tag="O", name=f"po{fc}")
                for fc in range(NF)
            ]
            for k in range(DO):
                for fc in range(NF):
                    nc.tensor.matmul(
                        pss[fc][:],
                        YT[:, k, it * P : (it + 1) * P],
                        Wo_sb[:, k, fc * 512 : (fc + 1) * 512],
                        start=(k == 0),
                        stop=(k == DO - 1),
                    )
            o_sb = opool.tile([P, D], F32, tag="osb", name="o_sb")
            for fc in range(NF):
                fsl = slice(fc * 512, (fc + 1) * 512)
                # scale on the scalar engine, bias-add on DVE
                nc.scalar.mul(o_sb[:, fsl], pss[fc][:], recip_sb[:, it : it + 1])
                nc.vector.tensor_add(o_sb[:, fsl], o_sb[:, fsl], bo_bcast[:, fsl])
            eng = nc.sync if it % 2 == 0 else nc.scalar
            eng.dma_start(out_r[:, it, :], o_sb[:])

    late.release()
    dram.release()
    const.release()


def _build():
    if "nc" in _CACHE:
        return _CACHE["nc"]
    import concourse.tile as tile
    import concourse.mybir as mybir
    from concourse import bacc

    nc = bacc.Bacc("TRN2", target_bir_lowering=False, debug=False, num_devices=8)
    F32 = mybir.dt.float32
    t = {}
    t["x"] = nc.dram_tensor("x", [S, D], F32, kind="ExternalInput").ap()
    for name in ("Wq", "Wk", "Wv", "Wo"):
        t[name] = nc.dram_tensor(name, [D, D], F32, kind="ExternalInput").ap()
    for name in ("bq", "bk", "bv", "bo"):
        t[name] = nc.dram_tensor(name, [D], F32, kind="ExternalInput").ap()
    t["out"] = nc.dram_tensor("out", [S, D], F32, kind="ExternalOutput").ap()

    with tile.TileContext(nc) as tc:
        _emit_body(nc, tc, t)
    nc.compile()
    _CACHE["nc"] = nc
    return nc


def kernel(x, Wq, bq, Wk, bk, Wv, bv, Wo, bo, _trace=False):
    from concourse.bass_utils import run_bass_kernel_spmd

    nc = _build()
    x = np.ascontiguousarray(np.asarray(x, dtype=np.float32))
    shared = {
        "Wq": np.ascontiguousarray(np.asarray(Wq, dtype=np.float32)),
        "Wk": np.ascontiguousarray(np.asarray(Wk, dtype=np.float32)),
        "Wv": np.ascontiguousarray(np.asarray(Wv, dtype=np.float32)),
        "Wo": np.ascontiguousarray(np.asarray(Wo, dtype=np.float32)),
        "bq": np.ascontiguousarray(np.asarray(bq, dtype=np.float32)),
        "bk": np.ascontiguousarray(np.asarray(bk, dtype=np.float32)),
        "bv": np.ascontiguousarray(np.asarray(bv, dtype=np.float32)),
        "bo": np.ascontiguousarray(np.asarray(bo, dtype=np.float32)),
    }
    in_maps = [{"x": x[b], **shared} for b in range(B)]
    res = run_bass_kernel_spmd(
        nc, in_maps, core_ids=list(range(B)), trace=_trace
    )
    out = np.stack([r["out"] for r in res.results], axis=0)
    if _trace:
        return out, res
    return out


# revision 29
# speedup vs baseline: 1.2012x; 1.0013x over previous
"""Single-head self-attention (B=8, S=2048, D=1024) on 8 TRN2 NeuronCores.

Data-parallel over batch: core b computes attention for x[b].
All compute in bf16 matmuls with fp32 PSUM accumulation; softmax in fp32.

Prologue avoids the DRAM round-trip cast: x streams fp32 over HWDGE in
row-chunks, DVE casts to bf16, the (otherwise idle) PE transposes each
chunk's eight 128x128 tiles into a single PSUM bank, and the scalar
engine copies them out to the persistent x^T. Weights other than Wq load
via SWDGE cast; Wq (on the critical path) streams fp32 over HWDGE with a
DVE cast.
"""

import sys

sys.path.insert(0, "/opt/trn_rl_repo")

import numpy as np

B, S, D = 8, 2048, 1024
P = 128
SO = S // P  # 16 s-tiles
DO = D // P  # 8 d-tiles
IC = 512  # i-chunk (query chunk) width
NIC = S // IC  # 4
NF = D // 512  # 2 free-dim chunks for D-wide outputs
NCH = S // P  # 16 x row-chunks

_CACHE = {}


def _emit_body(nc, tc, t):
    import concourse.mybir as mybir
    from concourse.masks import make_identity

    F32 = mybir.dt.float32
    BF16 = mybir.dt.bfloat16
    Exp = mybir.ActivationFunctionType.Exp
    Ident = mybir.ActivationFunctionType.Identity

    const = tc.alloc_tile_pool(name="const", bufs=1)
    dram = tc.alloc_tile_pool(name="dram", bufs=1, space="DRAM")

    # ---- weights, all on SWDGE (cast fp32->bf16 in the DMA). Wq first and
    # alone (it gates the first projections); fences keep later weights from
    # stealing SWDGE bandwidth via round-robin.
    wqkv = tc.alloc_tile_pool(name="wqkv", bufs=1)
    W_sb = {}
    for name in ("Wq", "Wk", "Wv"):
        W_sb[name] = wqkv.tile([P, DO, D], BF16, name=f"{name}_sb")

    def load_w(name):
        nc.gpsimd.dma_start(
            W_sb[name][:], t[name].rearrange("(ko ki) e -> ki ko e", ki=P)
        )

    w_r = {n: t[n].rearrange("(ko ki) e -> ki ko e", ki=P) for n in ("Wq",)}
    fence_d = dram.tile([4, 16], BF16, name="fence_d")

    # Wq in two halves with a fence between: the k0-3 half lands ~10us
    # earlier than a monolithic load would, unblocking the first Q-proj
    # k-loops (SWDGE round-robins in-flight DMAs, hence the fences).
    nc.gpsimd.dma_start(W_sb["Wq"][:, 0:4, :], w_r["Wq"][:, 0:4, :])

    # identity for PE transposes (gpsimd compute overlaps the Wq DMA)
    identity = const.tile([P, P], BF16, name="ident")
    make_identity(nc, identity[:])

    # bias row pre-cast to bf16 by a tiny SWDGE cast-DMA (keeps the DVE
    # stream free of casts whose inputs arrive late)
    bv_row_bf = const.tile([1, D], BF16, name="bv_row_bf")
    nc.gpsimd.dma_start(bv_row_bf[:], t["bv"].rearrange("(a d) -> a d", a=1))

    nc.gpsimd.dma_start(fence_d[0:1, :], W_sb["Wq"][0:1, 3, 1008:1024])
    nc.gpsimd.dma_start(W_sb["Wq"][:, 4:8, :], w_r["Wq"][:, 4:8, :])
    nc.gpsimd.dma_start(fence_d[1:2, :], W_sb["Wq"][0:1, 7, 1008:1024])
    load_w("Wk")
    load_w("Wv")
    nc.gpsimd.dma_start(fence_d[2:3, :], W_sb["Wv"][0:1, 7, 1008:1024])

    ones_row = const.tile([1, P], BF16, name="ones_row")
    nc.vector.memset(ones_row[:], 1.0)
    ones_j = const.tile([P, 1], BF16, name="ones_j")
    nc.vector.memset(ones_j[:], 1.0)

    # per-partition biases for Q^T/K^T (e on partitions); tiny loads on the
    # scalar HWDGE queue ahead of the x chunks.
    bq_sb = const.tile([P, DO], F32, name="bq_sb")
    nc.scalar.dma_start(bq_sb[:], t["bq"].rearrange("(eo ei) -> ei eo", ei=P))
    bk_sb = const.tile([P, DO], F32, name="bk_sb")
    nc.scalar.dma_start(bk_sb[:], t["bk"].rearrange("(eo ei) -> ei eo", ei=P))

    # ---- persistent activations
    xt_pool = tc.alloc_tile_pool(name="xt_pool", bufs=1)
    xT = xt_pool.tile([P, DO, S], BF16, name="xT")  # [d_inner, d_outer, s]
    QT = const.tile([P, DO, S], BF16, name="QT")  # [e_i, e_o, s]
    KT = const.tile([P, DO, S], BF16, name="KT")
    V = const.tile([P, SO, D], BF16, name="V")  # [s_i, s_o, e]
    recip_sb = const.tile([P, SO], F32, name="recip_sb")

    # bv_bcast only lives until the V drains; park it in xt_pool so it
    # frees with the projections.
    bv_bcast = xt_pool.tile([P, D], F32, name="bv_bcast")

    def emit_qk_proj(Wn, b_sb, OUT, sc, ppsum):
        # lhsT = W tile [d, e-tile] (stationary), rhs = xT [d, s-chunk]
        for eo in range(DO):
            pp = ppsum.tile([P, 512], F32, tag="proj", name="pp")
            for k in range(DO):
                nc.tensor.matmul(
                    pp[:],
                    W_sb[Wn][:, k, eo * P : (eo + 1) * P],
                    xT[:, k, sc * 512 : (sc + 1) * 512],
                    start=(k == 0),
                    stop=(k == DO - 1),
                )
            nc.scalar.activation(
                OUT[:, eo, sc * 512 : (sc + 1) * 512],
                pp[:],
                Ident,
                bias=b_sb[:, eo : eo + 1],
            )

    # ---- prologue: stream x fp32 over both HWDGE queues, DVE-cast to bf16,
    # PE-transpose each chunk's 8 k-tiles into one PSUM bank. Q-proj s-chunks
    # are interleaved so the PE starts projecting as soon as Wq + the first
    # 4 chunks are in.
    with tc.tile_pool(name="stage", bufs=3) as stage, \
         tc.tile_pool(name="bstage", bufs=4) as bstage, \
         tc.tile_pool(name="tpsum", bufs=2, space="PSUM") as tpsum, \
         tc.tile_pool(name="ppsum", bufs=4, space="PSUM") as ppsum:
        # all x chunks on the (otherwise idle) sync queue; the in-order queue
        # plus stage-pool WAR deps pipeline the stream 3 chunks deep without
        # ever blocking the scalar engine on a dispatch
        for c in range(NCH):
            st = stage.tile([P, D], F32, tag="xs", name="xs")
            nc.sync.dma_start(st[:], t["x"][c * P : (c + 1) * P, :])
            bt = bstage.tile([P, D], BF16, tag="xb", name="xb")
            # alternate the cast/copy engines per chunk so neither engine
            # serializes the chunk pipeline
            if c % 2 == 0:
                nc.vector.tensor_copy(bt[:], st[:])  # fp32 -> bf16
            else:
                nc.scalar.copy(bt[:], st[:])
            # 8 transposes into one PSUM bank as a single accumulation
            # group (disjoint byte ranges; one zero-region mark).
            ps = tpsum.tile([P, DO, P], BF16, tag="tp", name="tp")
            for j in range(DO):
                nc.tensor.matmul(
                    ps[:, j, :],
                    bt[:, j * P : (j + 1) * P],
                    identity[:],
                    start=(j == 0),
                    stop=(j == DO - 1),
                    is_transpose=True,
                )
            eng_copy = (
                nc.scalar.copy if c % 2 == 0 else nc.vector.tensor_copy
            )
            eng_copy(xT[:, :, c * P : (c + 1) * P], ps[:])
            if c % 4 == 3:
                emit_qk_proj("Wq", bq_sb, QT, c // 4, ppsum)

        for sc in range(4):
            emit_qk_proj("Wk", bk_sb, KT, sc, ppsum)

    with tc.tile_pool(name="vpsum", bufs=4, space="PSUM") as vpsum:
        # bv broadcast (bf16 rhs so the tiny matmuls run at full rate)
        for fc in range(NF):
            psb = vpsum.tile([P, 512], F32, tag="proj", name="bps")
            nc.tensor.matmul(
                psb[:], ones_row[:], bv_row_bf[:, fc * 512 : (fc + 1) * 512],
                start=True, stop=True,
            )
            nc.vector.tensor_copy(bv_bcast[:, fc * 512 : (fc + 1) * 512], psb[:])

        # V: lhsT = xT tile [d, s-tile], rhs = Wv [d, e-chunk]
        for so in range(SO):
            pss = [
                vpsum.tile([P, 512], F32, tag="proj", name=f"pv{fc}")
                for fc in range(NF)
            ]
            for k in range(DO):
                for fc in range(NF):
                    nc.tensor.matmul(
                        pss[fc][:],
                        xT[:, k, so * P : (so + 1) * P],
                        W_sb["Wv"][:, k, fc * 512 : (fc + 1) * 512],
                        start=(k == 0),
                        stop=(k == DO - 1),
                    )
            for fc in range(NF):
                nc.vector.tensor_add(
                    V[:, so, fc * 512 : (fc + 1) * 512],
                    pss[fc][:],
                    bv_bcast[:, fc * 512 : (fc + 1) * 512],
                )

    # projections done: reclaim Wq/Wk/Wv and xT space for YT, Wo
    xt_pool.release()
    wqkv.release()

    late = tc.alloc_tile_pool(name="late", bufs=1)
    YT = late.tile([P, DO, S], BF16, name="YT")  # [e_i, e_o, i]
    Wo_sb = late.tile([P, DO, D], BF16, name="Wo_sb")
    bo_row_bf = late.tile([1, D], BF16, name="bo_row_bf")
    nc.gpsimd.dma_start(bo_row_bf[:], t["bo"].rearrange("(a d) -> a d", a=1))
    nc.gpsimd.dma_start(
        Wo_sb[:], t["Wo"].rearrange("(ko ki) e -> ki ko e", ki=P)
    )
    bo_bcast = late.tile([P, D], F32, name="bo_bcast")

    cs_dram = dram.tile([S], F32)
    cs_dram_2d = cs_dram.rearrange("(a s) -> a s", a=1)
    cs_dram_cols = cs_dram.rearrange("(io ii) -> ii io", ii=P)

    # ---- attention. PE stream interleave: S(0) S(1) cs(0) Y(0) S(2)
    # cs(1) Y(1) S(3) cs(2) Y(2) cs(3) Y(3) — Y(ic)'s dependency on all of
    # E(ic) is hidden behind S(ic+1), so the PE never waits on exp.
    inv_sqrt_d = float(1.0 / np.sqrt(D))
    with tc.tile_pool(name="epool", bufs=2) as epool, \
         tc.tile_pool(name="red_pool", bufs=1) as red_pool, \
         tc.tile_pool(name="csb_pool", bufs=2) as csb_pool, \
         tc.tile_pool(name="spsum", bufs=3, space="PSUM") as spsum, \
         tc.tile_pool(name="cpsum", bufs=2, space="PSUM") as cpsum, \
         tc.tile_pool(name="ypsum", bufs=3, space="PSUM") as ypsum:
        E = {}

        def emit_scores(ic):
            isl = slice(ic * IC, (ic + 1) * IC)
            E[ic] = epool.tile([P, SO, IC], BF16, tag="E", name="E")
            for jt in range(SO):
                ps = spsum.tile([P, IC], F32, tag="S", name="sps")
                for k in range(DO):
                    nc.tensor.matmul(
                        ps[:],
                        KT[:, k, jt * P : (jt + 1) * P],
                        QT[:, k, isl],
                        start=(k == 0),
                        stop=(k == DO - 1),
                    )
                nc.scalar.activation(
                    E[ic][:, jt, :], ps[:], Exp, scale=inv_sqrt_d
                )

        def emit_cs(ic):
            # softmax denominators: DVE folds 16 E planes down to 4, then
            # 4 ones-matmuls finish the partition reduction.
            isl = slice(ic * IC, (ic + 1) * IC)
            red = red_pool.tile([P, 12, IC], BF16, tag="red", name="red")
            nc.vector.tensor_add(
                red[:, 0:8, :], E[ic][:, 0:8, :], E[ic][:, 8:16, :]
            )
            nc.vector.tensor_add(
                red[:, 8:12, :], red[:, 0:4, :], red[:, 4:8, :]
            )
            cs = cpsum.tile([1, IC], F32, tag="cs", name="cs")
            for tt in range(4):
                nc.tensor.matmul(
                    cs[:], ones_j[:], red[:, 8 + tt, :],
                    start=(tt == 0), stop=(tt == 3),
                )
            csb = csb_pool.tile([1, IC], F32, tag="csb", name="csb")
            nc.vector.tensor_copy(csb[:], cs[:])
            nc.sync.dma_start(cs_dram_2d[:, isl], csb[:])
            # round-trip this chunk's denominators to per-partition layout
            # and invert now, so the output drains never wait on recip
            nc.sync.dma_start(
                recip_sb[:, ic * 4 : (ic + 1) * 4],
                cs_dram_cols[:, ic * 4 : (ic + 1) * 4],
            )
            nc.vector.reciprocal(
                recip_sb[:, ic * 4 : (ic + 1) * 4],
                recip_sb[:, ic * 4 : (ic + 1) * 4],
            )

        def emit_Y(ic):
            isl = slice(ic * IC, (ic + 1) * IC)
            for eo in range(DO):
                py = ypsum.tile([P, IC], F32, tag="Y", name="yps")
                for jt in range(SO):
                    nc.tensor.matmul(
                        py[:],
                        V[:, jt, eo * P : (eo + 1) * P],
                        E[ic][:, jt, :],
                        start=(jt == 0),
                        stop=(jt == SO - 1),
                    )
                nc.vector.tensor_copy(YT[:, eo, isl], py[:])

        emit_scores(0)
        # bo broadcast, tucked behind S(0) so neither the PE nor DVE waits
        # (bo_row_bf lands off the SWDGE queue long before this)
        for fc in range(NF):
            psb = spsum.tile([P, IC], F32, tag="S", name="bops")
            nc.tensor.matmul(
                psb[:], ones_row[:], bo_row_bf[:, fc * 512 : (fc + 1) * 512],
                start=True, stop=True,
            )
            nc.vector.tensor_copy(bo_bcast[:, fc * 512 : (fc + 1) * 512], psb[:])
        emit_scores(1)
        emit_cs(0)
        emit_Y(0)
        emit_scores(2)
        emit_cs(1)
        emit_Y(1)
        emit_scores(3)
        emit_cs(2)
        emit_Y(2)
        emit_cs(3)
        emit_Y(3)

    # ---- output projection: out = (Y^T.T @ Wo) * recip + bo
    out_r = t["out"].rearrange("(so si) f -> si so f", si=P)
    with tc.tile_pool(name="opool", bufs=3) as opool, \
         tc.tile_pool(name="opsum", bufs=4, space="PSUM") as opsum:
        for it in range(SO):
            pss = [
                opsum.tile([P, 512], F32, tag="O", name=f"po{fc}")
                for fc in range(NF)
            ]
            for k in range(DO):
                for fc in range(NF):
                    nc.tensor.matmul(
                        pss[fc][:],
                        YT[:, k, it * P : (it + 1) * P],
                        Wo_sb[:, k, fc * 512 : (fc + 1) * 512],
                        start=(k == 0),
                        stop=(k == DO - 1),
                    )
            o_sb = opool.tile([P, D], F32, tag="osb", name="o_sb")
            eng = nc.sync if it % 2 == 0 else nc.scalar
            for fc in range(NF):
                fsl = slice(fc * 512, (fc + 1) * 512)
                # scale on the scalar engine, bias-add on DVE; DMA each
                # half as soon as it's ready to shorten the drain tail
                nc.scalar.mul(o_sb[:, fsl], pss[fc][:], recip_sb[:, it : it + 1])
                nc.vector.tensor_add(o_sb[:, fsl], o_sb[:, fsl], bo_bcast[:, fsl])
                eng.dma_start(out_r[:, it, fsl], o_sb[:, fsl])

    late.release()
    dram.release()
    const.release()


def _build():
    if "nc" in _CACHE:
        return _CACHE["nc"]
    import concourse.tile as tile
    import concourse.mybir as mybir
    from concourse import bacc

    nc = bacc.Bacc("TRN2", target_bir_lowering=False, debug=False, num_devices=8)
    F32 = mybir.dt.float32
    t = {}
    t["x"] = nc.dram_tensor("x", [S, D], F32, kind="ExternalInput").ap()
    for name in ("Wq", "Wk", "Wv", "Wo"):
        t[name] = nc.dram_tensor(name, [D, D], F32, kind="ExternalInput").ap()
    for name in ("bq", "bk", "bv", "bo"):
        t[name] = nc.dram_tensor(name, [D], F32, kind="ExternalInput").ap()
    t["out"] = nc.dram_tensor("out", [S, D], F32, kind="ExternalOutput").ap()

    with tile.TileContext(nc) as tc:
        _emit_body(nc, tc, t)
    nc.compile()
    _CACHE["nc"] = nc
    return nc


def kernel(x, Wq, bq, Wk, bk, Wv, bv, Wo, bo, _trace=False):
    from concourse.bass_utils import run_bass_kernel_spmd

    nc = _build()
    x = np.ascontiguousarray(np.asarray(x, dtype=np.float32))
    shared = {
        "Wq": np.ascontiguousarray(np.asarray(Wq, dtype=np.float32)),
        "Wk": np.ascontiguousarray(np.asarray(Wk, dtype=np.float32)),
        "Wv": np.ascontiguousarray(np.asarray(Wv, dtype=np.float32)),
        "Wo": np.ascontiguousarray(np.asarray(Wo, dtype=np.float32)),
        "bq": np.ascontiguousarray(np.asarray(bq, dtype=np.float32)),
        "bk": np.ascontiguousarray(np.asarray(bk, dtype=np.float32)),
        "bv": np.ascontiguousarray(np.asarray(bv, dtype=np.float32)),
        "bo": np.ascontiguousarray(np.asarray(bo, dtype=np.float32)),
    }
    in_maps = [{"x": x[b], **shared} for b in range(B)]
    res = run_bass_kernel_spmd(
        nc, in_maps, core_ids=list(range(B)), trace=_trace
    )
    out = np.stack([r["out"] for r in res.results], axis=0)
    if _trace:
        return out, res
    return out
